# revision 50
# baseline (speedup 1.0000x reference)
"""Distributed Trainium2 Bass kernel for GQA causal attention
(S=2048, DIM=4096, NH=32, NKV=8, HD=128), tensor-parallel over heads on 8
NeuronCores. ~508us HW (vs 645us phase-separated baseline).

Per-core program (core c owns q-heads 4c..4c+3 and kv-head c), emitted as
ONE fused software-pipelined stream so the PE never phase-barriers:

  1. QKV projection, m-outer (each 128-row output block finishes its full
     32-tile contraction before the next starts; 2 PSUM banks suffice).
     Host supplies [p, m, kb, col] weights and [p, si, kb, s] x so every
     DMA line is contiguous per partition. All secondary loads ride the
     Scalar engine's hardware DMA queue in dependency order.
  2. RoPE: signed pair-permutation matmul (bf16) + DVE combine, injected
     into the next m-block's matmul stream; v PE-transposed likewise.
  3. Attention for q-tile qt is interleaved into QKV tile si=qt+1's
     matmul stream at a fixed pace (ATT_PACE), with a depth-2 pipeline:
     scores matmul + exp of task i+2 are emitted before pv of task i, so
     the ACT-engine exp latency never blocks the PE. Diagonal kv-blocks
     shrink the moving window to [128d, 512); one shared [128,128]
     triangular mask is applied in place on 128 columns. The softmax
     denominator is accumulated on the DVE (f32r) and reduced by a
     single ones-column matmul per (head, qt).
  4. Per-(qt,head) normalize (reciprocal on [128,4] via reshaping DMAs,
     partition-broadcast by a tiny f32r matmul) is deferred ~8 tasks to
     hide latency; AllGathers fire as halves complete: per-qt 4MB
     gathers for qt=0..2, then a 2MB half + two 1MB per-head gathers
     for qt=3 so the last collectives are small and fully overlapped.
  5. Output projection consumes gathered y in 8-kblock groups (wo and y
     strips DMA-prefetched one group ahead), scheduled at attention head
     boundaries per OP_SCHED so each group lands just after its
     AllGather completes; partial sums accumulate on the DVE into SBUF.

PSUM budget (8 banks): A2 qkv accum + normalize-bcast, B2 outproj pairs,
C2 scores/rot/v-transpose/denominator, D2 attention y accum.
"""

import sys

sys.path.insert(0, "/opt/trn_rl_repo")

import numpy as np
import ml_dtypes

import concourse.bass as bass
import concourse.mybir as mybir
import concourse.tile as tile
from concourse import bacc
from concourse import bass_utils
from concourse.bass import broadcast_tensor_aps

S, DIM = 2048, 4096
NH, NKV, HD = 32, 8, 128
NCORES = 8
QH = NH // NCORES  # 4 q heads per core
KT = DIM // 128  # 32 contraction tiles
ST = S // 512  # 4 sequence tiles of 512
SCALE = 1.0 / float(np.sqrt(HD))

BF = mybir.dt.bfloat16
F32 = mybir.dt.float32
F32R = mybir.dt.float32r
F8 = mybir.dt.float8e4
ALU = mybir.AluOpType
ACTF = mybir.ActivationFunctionType

USE_DMA_TRANSPOSE = False


def r32(ap):
    return ap.bitcast(F32R)


def block_list(qt):
    """Block order for one (head, qt): off-diagonals ascending (first is
    full width and carries the accumulation start flag), then diagonals
    d=0..3 with shrinking q windows. Entries: (j, qoff, width, is_diag)."""
    bl = [(j, 0, 512, False) for j in range(4 * qt)]
    bl += [(4 * qt + d, 128 * d, 512 - 128 * d, True) for d in (0, 1, 2, 3)]
    return bl


# outproj groups (si, half, core-group) emitted after attention head
# (qt, h) completes (ascending qt); group needs AllGather(si, half) done.
OP_SCHED = {
    (2, 1): [("qt", 0, 0)],
    (2, 2): [("qt", 0, 1)],
    (2, 3): [("qt", 0, 2)],
    (3, 0): [("qt", 0, 3), ("qt", 1, 0)],
    (3, 1): [("qt", 1, 1), ("qt", 1, 2)],
    (3, 2): [("qt", 1, 3), ("qt", 2, 0)],
    (3, 3): [("qt", 2, 1), ("qt", 2, 2)],
    "end": [
        ("qt", 2, 3),
        ("half", 0, 0), ("half", 0, 1),
        ("head", 2), ("head", 3),
    ],
}
# attention-slot pacing: one attention task per N QKV matmuls, per si window
ATT_PACE = {1: 8, 2: 4, 3: 3}


def build_nc():
    nc = bacc.Bacc(
        "TRN2",
        target_bir_lowering=False,
        debug=False,
        enable_asserts=True,
        num_devices=NCORES,
    )

    xt = nc.dram_tensor("xt", [128, ST * KT * 512], BF, kind="ExternalInput").ap()
    wqkvt = nc.dram_tensor("wqkvt", [128, 6 * KT * 128], BF, kind="ExternalInput").ap()
    wot = nc.dram_tensor("wot", [128, KT * 512], BF, kind="ExternalInput").ap()
    cost = nc.dram_tensor("cost", [128, S], BF, kind="ExternalInput").ap()
    sint = nc.dram_tensor("sint", [128, S], BF, kind="ExternalInput").ap()
    trit = nc.dram_tensor("trit", [128, 128], BF, kind="ExternalInput").ap()
    rpermt = nc.dram_tensor("rpermt", [128, 128], BF, kind="ExternalInput").ap()
    identt = nc.dram_tensor("identt", [128, 128], F32, kind="ExternalInput").ap()
    onescolt = nc.dram_tensor("onescolt", [1, 128], F32R, kind="ExternalInput").ap()
    onesvt = nc.dram_tensor("onesvt", [128, 1], BF, kind="ExternalInput").ap()
    onesmt = nc.dram_tensor("onesmt", [128, 128], BF, kind="ExternalInput").ap()
    outt = nc.dram_tensor("outt", [512, S], F32, kind="ExternalOutput").ap()

    xt_r = xt.rearrange("p (si kb s) -> p si kb s", si=ST, kb=KT)
    wqkvt_r = wqkvt.rearrange("p (m kb c) -> p m kb c", m=6, kb=KT)
    wot_r = wot.rearrange("p (kb c) -> p kb c", kb=KT)
    wot_h = wot.rearrange("p (c four col) -> p c four col", four=4, col=512)
    # qt3 half-group view: kb = 16*cg + 4*ci2 + 2*hf + hh
    wot_q3 = wot.rearrange(
        "p (cg ci2 hf hh c) -> p cg ci2 hf hh c", cg=2, ci2=4, hf=2, hh=2
    )
    outt_r = outt.rearrange("(oc p) s -> p oc s", p=128)

    with tile.TileContext(nc) as tc:
        with (
            tc.tile_pool(name="const", bufs=1) as const,
            tc.tile_pool(name="qkvsb", bufs=1) as qkvsb,
            tc.tile_pool(name="wqkv", bufs=1) as wqkv,
            tc.tile_pool(name="xs", bufs=1) as xs,
            tc.tile_pool(name="stg", bufs=1) as stg,
            tc.tile_pool(name="pp", bufs=1) as pp,
            tc.tile_pool(name="nrm", bufs=1) as nrm,
            tc.tile_pool(name="ys", bufs=1) as ys,
            tc.tile_pool(name="osb", bufs=1) as osb,
            tc.tile_pool(name="ps", bufs=8, space="PSUM") as ps,
            tc.tile_pool(name="dram", bufs=1, space="DRAM") as dram,
        ):
            tri_sb = const.tile([128, 128], BF)
            rperm_sb = const.tile([128, 128], BF)
            ident_sb = const.tile([128, 128], F32)
            onescol_sb = const.tile([1, 128], F32R)
            onescol2_sb = const.tile([1, 128], F32R)
            onesv_sb = const.tile([128, 1], BF)
            cos_sb = const.tile([128, S], BF)
            sin_sb = const.tile([128, S], BF)

            # persistent activations, attention operands in bf16
            q_sb = qkvsb.tile([128, QH, S], BF)  # rope'd qT, head-major
            k_sb = qkvsb.tile([128, S], BF)  # rope'd kT
            # v, block-transposed, 128 cols per kv-block
            v_sb = qkvsb.tile([128, S], BF)
            w_sb = wqkv.tile([128, 6, KT, 128], BF)

            # PSUM tags: A = qkv accum + normalize-bcast, B = outproj pairs,
            # C = scores / rope-rot / v-transpose, D = attention y accum.
            def A(name):
                return ps.tile([128, 512], F32, tag="A", bufs=2, name=name)

            def B(name):
                return ps.tile([128, 512], F32, tag="B", bufs=2, name=name)

            def C(name, shape=(128, 512)):
                return ps.tile(list(shape), F32, tag="C", bufs=2, name=name)

            # ---- DMA startup -------------------------------------------------
            x_tiles = {}

            def load_x(si, half=None):
                if si not in x_tiles:
                    x_tiles[si] = xs.tile(
                        [128, KT, 512], BF, tag="x", bufs=2, name=f"x{si}"
                    )
                t = x_tiles[si]
                for k4 in range(KT // 4):
                    nc.sync.dma_start(
                        t[:, 4 * k4 : 4 * k4 + 4, :],
                        xt_r[:, si, 4 * k4 : 4 * k4 + 4, :],
                    )

            load_x(0)
            nc.scalar.dma_start(w_sb[:, 0], wqkvt_r[:, 0])
            nc.scalar.dma_start(w_sb[:, 1], wqkvt_r[:, 1])
            nc.scalar.dma_start(cos_sb, cost)
            nc.scalar.dma_start(sin_sb, sint)
            nc.scalar.dma_start(tri_sb, trit)
            nc.scalar.dma_start(rperm_sb, rpermt)
            nc.scalar.dma_start(ident_sb, identt)
            nc.scalar.dma_start(onescol2_sb, onescolt)
            nc.scalar.dma_start(onesv_sb, onesvt)
            for m in range(2, 6):
                nc.scalar.dma_start(w_sb[:, m], wqkvt_r[:, m])

            # ---- rope / v epilogues -----------------------------------------
            def make_rope(si, m, src_ps):
                s0 = 512 * si

                def fire():
                    stage = stg.tile([128, 512], BF, tag="stage", bufs=1)
                    nc.scalar.copy(stage, src_ps)
                    rot = C(f"rot_{si}_{m}")
                    nc.tensor.matmul(rot, rperm_sb, stage)
                    dst = (
                        q_sb[:, m, s0 : s0 + 512]
                        if m < QH
                        else k_sb[:, s0 : s0 + 512]
                    )
                    t1 = stg.tile([128, 512], BF, tag="t1", bufs=1)
                    nc.vector.tensor_tensor(
                        t1, stage, cos_sb[:, s0 : s0 + 512], ALU.mult
                    )
                    t2 = stg.tile([128, 512], BF, tag="t2", bufs=1)
                    nc.vector.tensor_tensor(
                        t2, rot, sin_sb[:, s0 : s0 + 512], ALU.mult
                    )
                    nc.vector.tensor_tensor(dst, t1, t2, ALU.add)

                return fire

            def make_v(si, src_ps):
                def fire():
                    vstage = stg.tile([128, 512], F32, tag="vstage", bufs=1)
                    nc.scalar.copy(vstage, src_ps)
                    for jj in range(4):
                        j = 4 * si + jj
                        vt_ps = C(f"vt_{si}_{jj}", (128, 128))
                        nc.tensor.transpose(
                            vt_ps, vstage[:, 128 * jj : 128 * (jj + 1)], ident_sb
                        )
                        nc.vector.tensor_copy(
                            v_sb[:, 128 * j : 128 * (j + 1)], vt_ps
                        )

                return fire

            # ---- gathered y buffers -----------------------------------------
            y_bounce = {}
            y_gather = {}
            for qt in range(ST - 1):
                y_bounce[qt] = dram.tile(
                    [512, 512], BF, tag=f"yb{qt}", name=f"ybounce{qt}"
                )
                y_gather[qt] = dram.tile(
                    [NCORES * 512, 512],
                    BF,
                    addr_space="Shared",
                    tag=f"yg{qt}",
                    name=f"ygather{qt}",
                )
            yb3h = dram.tile([256, 512], BF, tag="yb3h", name="ybounce3_h01")
            yg3h = dram.tile(
                [NCORES * 256, 512], BF, addr_space="Shared",
                tag="yg3h", name="ygather3_h01",
            )
            yb3s = {}
            yg3s = {}
            for h in (2, 3):
                yb3s[h] = dram.tile([128, 512], BF, tag=f"yb3s{h}",
                                    name=f"ybounce3s{h}")
                yg3s[h] = dram.tile(
                    [NCORES * 128, 512], BF, addr_space="Shared",
                    tag=f"yg3s{h}", name=f"ygather3s{h}",
                )
            # tiny warm-up collective: pays the first-collective handshake
            # cost while si=0 is DMA-bound
            wu_b = dram.tile([1, 128], BF, tag="wub", name="wu_b")
            wu_g = dram.tile([NCORES, 128], BF, addr_space="Shared",
                             tag="wug", name="wu_g")
            wu_src = nrm.tile([1, 128], BF, tag="wu", bufs=1)
            nc.vector.memset(wu_src, 0.0)
            nc.sync.dma_start(wu_b, wu_src)
            nc.gpsimd.collective_compute(
                "AllGather", ALU.bypass, ins=[wu_b.opt()], outs=[wu_g.opt()],
                replica_groups=[list(range(NCORES))],
            )

            # ---- outproj ----------------------------------------------------
            osb_acc = {}
            group_cnt = {}

            def prefetch_group(g):
                if g[0] == "qt":
                    _, si, cg = g
                    tagn = f"{si}_{cg}"
                    wo_g = ys.tile(
                        [128, 8, 512], BF, tag="wog", bufs=2, name=f"wog_{tagn}"
                    )
                    nc.sync.dma_start(wo_g, wot_r[:, 8 * cg : 8 * cg + 8, :])
                    ysrc = y_gather[si].opt().rearrange(
                        "(ch p) q -> p ch q", p=128
                    )[:, 8 * cg : 8 * cg + 8, :]
                elif g[0] == "half":
                    _, hf, cg = g
                    si = ST - 1
                    tagn = f"3h{hf}_{cg}"
                    wo_g4 = ys.tile(
                        [128, 4, 2, 512], BF, tag="wog", bufs=2,
                        name=f"wog_{tagn}",
                    )
                    nc.sync.dma_start(wo_g4, wot_q3[:, cg, :, hf, :, :])
                    wo_g = wo_g4
                    ysrc = yg3h.opt().rearrange(
                        "(ch p) q -> p ch q", p=128
                    )[:, 8 * cg : 8 * cg + 8, :]
                else:
                    _, h = g
                    si = ST - 1
                    tagn = f"3s{h}"
                    wo_g = ys.tile(
                        [128, 8, 512], BF, tag="wog", bufs=2, name=f"wog_{tagn}"
                    )
                    nc.sync.dma_start(wo_g, wot_h[:, :, h, :])
                    ysrc = yg3s[h].opt().rearrange("(c p) q -> p c q", p=128)
                ystrip = ys.tile(
                    [128, 8, 512], BF, tag="ystrip", bufs=2, name=f"ys_{tagn}"
                )
                nc.sync.dma_start(ystrip, ysrc)
                return (g, si, tagn, wo_g, ystrip)

            def compute_group(pg):
                g, si, tagn, wo_g, ystrip = pg
                n = group_cnt.get(si, 0)
                group_cnt[si] = n + 1
                if n == 0:
                    osb_acc[si] = osb.tile(
                        [128, 4, 512], F32, tag="osb", bufs=1, name=f"osb{si}"
                    )
                for pair in ((0, 1), (2, 3)):
                    pps = {oc: B(f"op_{tagn}_{oc}") for oc in pair}
                    for ci in range(8):
                        for oc in pair:
                            if g[0] == "half":
                                wsl = wo_g[:, ci // 2, ci % 2,
                                           128 * oc : 128 * (oc + 1)]
                            else:
                                wsl = wo_g[:, ci, 128 * oc : 128 * (oc + 1)]
                            nc.tensor.matmul(
                                pps[oc],
                                wsl,
                                ystrip[:, ci, :],
                                start=(ci == 0),
                                stop=(ci == 7),
                            )
                    for oc in pair:
                        if n == 0:
                            nc.vector.tensor_copy(osb_acc[si][:, oc, :], pps[oc])
                        else:
                            nc.vector.tensor_tensor(
                                osb_acc[si][:, oc, :],
                                osb_acc[si][:, oc, :],
                                pps[oc],
                                ALU.add,
                            )
                if n == 3:
                    s0 = 512 * si
                    for oc in range(4):
                        nc.sync.dma_start(
                            outt_r[:, oc, s0 : s0 + 512], osb_acc[si][:, oc, :]
                        )

            # ---- normalize --------------------------------------------------
            def normalize_tail(st):
                yraw, den, h, qt = st
                den_t = nrm.tile([128, 4], F32, tag="dent", bufs=2)
                nc.scalar.dma_start(den_t, den)
                rec_t = nrm.tile([128, 4], F32R, tag="rect", bufs=2)
                with nc.allow_low_precision(reason="f32r reciprocal"):
                    nc.vector.reciprocal(rec_t, den_t)
                rec = nrm.tile([1, 512], F32R, tag="rec", bufs=1)
                nc.scalar.dma_start(rec, rec_t)
                bc_ps = A(f"bc_{qt}_{h}")
                nc.tensor.matmul(bc_ps, onescol2_sb, rec)
                yn = nrm.tile([128, 512], BF, tag="yn", bufs=1)
                nc.vector.tensor_tensor(yn, yraw, bc_ps, ALU.mult)
                if qt == ST - 1:
                    if h <= 1:
                        nc.scalar.dma_start(
                            yb3h[128 * h : 128 * (h + 1), :], yn
                        )
                        if h == 1:
                            nc.gpsimd.collective_compute(
                                "AllGather",
                                ALU.bypass,
                                ins=[yb3h.opt()],
                                outs=[yg3h.opt()],
                                replica_groups=[list(range(NCORES))],
                            )
                    else:
                        nc.scalar.dma_start(yb3s[h], yn)
                        nc.gpsimd.collective_compute(
                            "AllGather",
                            ALU.bypass,
                            ins=[yb3s[h].opt()],
                            outs=[yg3s[h].opt()],
                            replica_groups=[list(range(NCORES))],
                        )
                else:
                    nc.scalar.dma_start(
                        y_bounce[qt][128 * h : 128 * (h + 1), :], yn
                    )
                    if h == QH - 1:
                        nc.gpsimd.collective_compute(
                            "AllGather",
                            ALU.bypass,
                            ins=[y_bounce[qt].opt()],
                            outs=[y_gather[qt].opt()],
                            replica_groups=[list(range(NCORES))],
                        )

            # ---- attention pipeline -----------------------------------------
            att_q = []
            inflight = []
            state = {}
            pend_norm = [None]
            pend_age = [0]
            pend_groups = [[]]
            tctr = [0]

            def push_qt(qt, part=None):
                bl = block_list(qt)
                for h in range(QH):
                    for bi, blk in enumerate(bl):
                        t = (qt, h, blk, bi == 0, bi == len(bl) - 1)
                        # part "off0": only head-0 off-diagonals (safe to run
                        # before v(qt) is staged); "rest": everything else
                        is_off0 = h == 0 and not blk[3]
                        if part == "off0" and not is_off0:
                            continue
                        if part == "rest" and is_off0:
                            continue
                        att_q.append(t)

            def emit_spe(t):
                qt, h, (j, qoff, w, diag), _, _ = t
                i = tctr[0]
                tctr[0] += 1
                s0 = 512 * qt
                sT = C(f"sT_{i}")
                nc.tensor.matmul(
                    sT[:, 0:w],
                    k_sb[:, 128 * j : 128 * (j + 1)],
                    q_sb[:, h, s0 + qoff : s0 + qoff + w],
                )
                p = pp.tile([128, 512], BF, tag="p", bufs=3, name=f"p_{i}")
                nc.scalar.activation(p[:, 0:w], sT[:, 0:w], ACTF.Exp, scale=SCALE)
                if diag:
                    nc.vector.tensor_tensor(
                        p[:, 0:128], p[:, 0:128], tri_sb, ALU.mult
                    )
                return p

            def emit_pv(t, p):
                qt, h, (j, qoff, w, diag), first, last = t
                if pend_norm[0] is not None:
                    pend_age[0] += 1
                    if pend_age[0] >= 8:
                        normalize_tail(pend_norm[0])
                        pend_norm[0] = None
                if (qt, h) not in state:
                    state[(qt, h)] = (
                        ps.tile([128, 512], F32, tag="D", bufs=2,
                                name=f"y_{qt}_{h}"),
                        nrm.tile([128, 512], BF, tag="dacc", bufs=2,
                                 name=f"dacc_{qt}_{h}"),
                    )
                y_ps, den_acc = state[(qt, h)]
                if first:
                    nc.vector.tensor_copy(den_acc, p)
                else:
                    nc.vector.tensor_tensor(
                        den_acc[:, qoff : qoff + w],
                        den_acc[:, qoff : qoff + w],
                        p[:, 0:w],
                        ALU.add,
                    )
                nc.tensor.matmul(
                    y_ps[:, qoff : qoff + w],
                    v_sb[:, 128 * j : 128 * (j + 1)],
                    p[:, 0:w],
                    start=first,
                    stop=last,
                    skip_group_check=True,
                )
                if last:
                    den_ps = C(f"den_{qt}_{h}")
                    nc.tensor.matmul(den_ps[0:1, :], onesv_sb, den_acc)
                    yraw = nrm.tile([128, 512], BF, tag="yraw", bufs=2,
                                    name=f"yraw_{qt}_{h}")
                    nc.scalar.copy(yraw, y_ps)
                    den = nrm.tile([1, 512], F32, tag="den", bufs=2,
                                   name=f"den_{qt}_{h}")
                    nc.vector.tensor_copy(den, den_ps[0:1, :])
                    if pend_norm[0] is not None:
                        normalize_tail(pend_norm[0])
                    pend_norm[0] = (yraw, den, h, qt)
                    pend_age[0] = 0
                    if (qt, h) == (ST - 1, QH - 1):
                        normalize_tail(pend_norm[0])  # eager final
                        pend_norm[0] = None
                    for pg in pend_groups[0]:
                        compute_group(pg)
                    pend_groups[0] = [
                        prefetch_group(g) for g in OP_SCHED.get((qt, h), [])
                    ]

            DEPTH = 2

            def att_slot():
                if not att_q:
                    return False
                t = att_q.pop(0)
                p = emit_spe(t)
                inflight.append((t, p))
                if len(inflight) > DEPTH:
                    emit_pv(*inflight.pop(0))
                return True

            # ---- fused QKV + attention + outproj ----------------------------
            pending = None
            for si in range(ST):
                pace = ATT_PACE.get(si, 0)
                mm = 0
                for m in range(6):
                    acc = A(f"qkv_{si}_{m}")
                    for k in range(KT):
                        if k == 3 and pending is not None:
                            pending()
                            pending = None
                        if k == 6 and m == 0 and si >= 1:
                            push_qt(si - 1)
                        if k == 6 and m == 4 and si == ST - 1:
                            push_qt(ST - 1, part="off0")
                        if k == 8 and m == 1 and si < ST - 1:
                            load_x(si + 1)
                        nc.tensor.matmul(
                            acc,
                            w_sb[:, m, k, :],
                            x_tiles[si][:, k, :],
                            start=(k == 0),
                            stop=(k == KT - 1),
                        )
                        mm += 1
                        if pace and mm % pace == 0:
                            att_slot()
                    pending = make_rope(si, m, acc) if m < 5 else make_v(si, acc)
            pending()  # v of last si
            push_qt(ST - 1, part="rest")

            # phase B: attention qt=3 + remaining outproj
            while att_slot():
                pass
            while inflight:
                emit_pv(*inflight.pop(0))
            for pg in pend_groups[0]:
                compute_group(pg)
            prev = []
            for g in OP_SCHED["end"]:
                pg = prefetch_group(g)
                for old_pg in prev:
                    compute_group(old_pg)
                prev = [pg]
            for old_pg in prev:
                compute_group(old_pg)

    nc.compile()
    return nc


def make_in_maps(x, freqs_cis, wq, wk, wv, wo):
    f32 = np.float32
    bf = ml_dtypes.bfloat16
    xT = np.ascontiguousarray(x.T)  # [DIM, S]
    xt2 = (
        xT.reshape(KT, 128, ST, 512).transpose(1, 2, 0, 3).reshape(128, -1)
    ).astype(bf)
    cos = np.ascontiguousarray(np.repeat(freqs_cis[:, :, 0].T, 2, axis=0)).astype(bf)
    sin = np.ascontiguousarray(np.repeat(freqs_cis[:, :, 1].T, 2, axis=0)).astype(bf)
    kvi = np.arange(128)[:, None]
    qi = np.arange(128)[None, :]
    tri = (kvi <= qi).astype(f32).astype(bf)  # [128,128]
    rperm = np.zeros((128, 128), f32)
    for r in range(64):
        rperm[2 * r, 2 * r + 1] = -1.0
        rperm[2 * r + 1, 2 * r] = 1.0
    rpermT = np.ascontiguousarray(rperm.T).astype(bf)
    ident = np.eye(128, dtype=f32)
    onescol = np.ones((1, 128), f32)
    onesv = np.ones((128, 1), bf)
    onesm = np.ones((128, 128), bf)

    in_maps = []
    for c in range(NCORES):
        W = np.concatenate(
            [
                wq[512 * c : 512 * (c + 1), :],
                wk[128 * c : 128 * (c + 1), :],
                wv[128 * c : 128 * (c + 1), :],
            ],
            axis=0,
        )  # [768, DIM]
        WT = np.ascontiguousarray(W.T)  # [DIM, 768]
        wqkv2 = (
            WT.reshape(KT, 128, 6, 128).transpose(1, 2, 0, 3).reshape(128, -1)
        ).astype(bf)
        wo_c = wo[512 * c : 512 * (c + 1), :]  # [512, DIM]
        WoT = np.ascontiguousarray(wo_c.T)  # [DIM, 512]
        wo2 = (
            WoT.reshape(KT, 128, 4, 128).transpose(1, 0, 2, 3).reshape(128, -1)
        ).astype(bf)
        in_maps.append(
            {
                "xt": np.ascontiguousarray(xt2),
                "wqkvt": np.ascontiguousarray(wqkv2),
                "wot": np.ascontiguousarray(wo2),
                "cost": cos,
                "sint": sin,
                "trit": np.ascontiguousarray(tri),
                "rpermt": rpermT,
                "identt": ident,
                "onescolt": onescol,
                "onesvt": onesv,
                "onesmt": onesm,
            }
        )
    return in_maps


def install_ntff_hook():
    """Inject the missing ``antenv.axon_hooks`` module backed by ctypes calls
    into libaxon_pjrt.so, enabling run_bass_kernel_spmd(trace=True) under
    axon. Also neuter upload_artifacts (no artifact bucket here)."""
    import sys as _sys
    import types
    import ctypes
    import contextlib

    if "antenv.axon_hooks" in _sys.modules:
        return
    so_path = "/opt/axon/libaxon_pjrt.so"
    lib = ctypes.CDLL(so_path)
    lib.axon_start_nrt_profile.argtypes = [
        ctypes.POINTER(ctypes.c_int64),
        ctypes.c_size_t,
    ]
    lib.axon_start_nrt_profile.restype = ctypes.c_int64
    lib.axon_stop_nrt_profile.argtypes = [ctypes.c_char_p]
    lib.axon_stop_nrt_profile.restype = ctypes.c_int64

    @contextlib.contextmanager
    def _hook(output_dir, device_ids):
        import jax

        jax.devices()
        if device_ids:
            ids = (ctypes.c_int64 * len(device_ids))(*device_ids)
            rc = lib.axon_start_nrt_profile(ids, len(device_ids))
        else:
            rc = lib.axon_start_nrt_profile(None, 0)
        if rc != 0:
            raise RuntimeError(f"axon_start_nrt_profile rc={rc}")
        try:
            yield
        finally:
            n = lib.axon_stop_nrt_profile(str(output_dir).encode())
            print(f"ntff profile: {n} file(s) written to {output_dir}")

    mod = types.ModuleType("antenv.axon_hooks")
    mod.get_axon_ntff_profile_hook = lambda: _hook
    mod.set_axon_ntff_profile_hook = lambda h: None
    _sys.modules["antenv.axon_hooks"] = mod
    import antenv

    antenv.axon_hooks = mod
    bass_utils.upload_artifacts = lambda tmpdir: tmpdir


def run(x, freqs_cis, wq, wk, wv, wo, trace=False, trace_kwargs=None):
    if trace:
        install_ntff_hook()
    nc = build_nc()
    in_maps = make_in_maps(x, freqs_cis, wq, wk, wv, wo)
    res = bass_utils.run_bass_kernel_spmd(
        nc,
        in_maps,
        core_ids=list(range(NCORES)),
        trace=trace,
        **(trace_kwargs or {}),
    )
    outs = [r["outt"] for r in res.results]  # each [512, S] = outT slice
    full = np.concatenate([np.asarray(o).T for o in outs], axis=1).astype(np.float32)
    return full, res


def kernel(x, freqs_cis, wq, wk, wv, wo):
    full, _ = run(
        np.asarray(x, np.float32),
        np.asarray(freqs_cis, np.float32),
        np.asarray(wq, np.float32),
        np.asarray(wk, np.float32),
        np.asarray(wv, np.float32),
        np.asarray(wo, np.float32),
    )
    return full
